# revision 12
# baseline (speedup 1.0000x reference)
"""MinLSTM cell for Trainium2 (Bass/Tile), v5: sigmoid + fused custom-DVE.

Data-parallel over batch on 8 cores (one row per core). PE floor is
~187us (3 bf16 GEMMs at 1 row/cycle @2.4GHz; fp8 fails the 2e-2 gate:
measured 3-4e-2 in numpy). Everything else exists to keep PE streaming
at 216ns/matmul with zero backpressure.

v4 lesson (trace): the exp/ln gate chain was 5 hops deep
(ACT ef/ei -> Pool s2 -> ACT ln2/rt -> DVE ut/bt -> scan); PSUM banks
freed slowly through that chain, stalling the PE ~250ns at group
boundaries, and the tail drained at ~4.1us/group. v5 shortens the
chain to 2 hops with the identity fp+ip = 1:

  - ACT (2 ops/tile): f = Sigmoid(psf + bf), i = Sigmoid(psi + bi),
    both bf16, straight from PSUM (frees psf/psi at lag 0).
  - DVE (3 ops/tile): at = f/(f+i) as ONE fused op (bitwise-NOT
    exponent-flip reciprocal seed + 1 Newton step, 7 ALU stages,
    ~0.17% max err); bt = (1-at)*(psh+bh) as ONE fused op (3 stages,
    reads PSUM directly -- frees psh at lag 1, no identity-ACT pass);
    then the hw scan. at = fp exactly because fp+ip = 1.
  - Pool/GpSimd: only DMA issue + weight loads (was a serialization
    link in the chain; now out of the loop entirely).
  - output hT [H,T] bf16 (halves store traffic); host converts.
"""

import sys

if "/opt/trn_rl_repo" not in sys.path:
    sys.path.insert(0, "/opt/trn_rl_repo")

import numpy as np
import ml_dtypes

B, T, D, H = 8, 4096, 768, 768
TC = 512                    # time-chunk (one PSUM bank of fp32)
NT = T // TC                # 8 chunks
KD = D // 128               # 6 bf16 K-tiles
MH = H // 128               # 6 hidden tiles

# Chebyshev-minimax seed constants for x*bitcast(~x) in [-4.5, -4]
# (from concourse.dve_ops.RECIP_APPROX_FAST_CONSTS).
_RC0, _RC1 = -0.23549792, 2.0017324

_state = {}


def _register_custom_ops():
    """Register the two fused DVE ops in concourse.dve_ops' registry so
    dve_table_for_ops can build the per-NEFF table for them."""
    import concourse.dve_ops as dops
    from concourse.dve_spec import Spec, Src0, Src1, C0, C1, AluOp, Bin, lower, _has_src1
    from concourse.dve_uop import DveOpSpec

    if "AT_FRAC_ANT" in dops._SUB_OPCODE_FOR_NAME:
        return

    def _recip1(x):
        x = x.astype(np.float32)
        nx = (~x.view(np.int32)).view(np.float32)
        y0 = (nx * np.float32(_RC0)).astype(np.float32)
        return (y0 * (np.float32(_RC1) - (x * y0).astype(np.float32))).astype(np.float32)

    # at = Src1 / (Src0 + Src1)  (call with in0=i, in1=f -> at = f/(f+i))
    x = Src0 + Src1
    nx = Bin(AluOp.BITWISE_NOT, x, x)
    y0 = nx * C0
    y1 = y0 * (C1 - x * y0)
    at_spec = Spec(
        body=Src1 * y1,
        reference=lambda in0, in1, s0, s1, imm2: (
            in1.astype(np.float32) * _recip1(in0.astype(np.float32) + in1)
        ),
    )
    # bt = (C0 - Src0) * (Src1 + C1)  (in0=at, in1=psh, s0=1.0, s1=bh)
    bt_spec = Spec(
        body=(C0 - Src0) * (Src1 + C1),
        reference=lambda in0, in1, s0, s1, imm2: (
            (s0 - in0.astype(np.float32)) * (in1 + s1)
        ),
    )

    for name, spec in (("AT_FRAC_ANT", at_spec), ("BT_FUSE_ANT", bt_spec)):
        row = max(dops._SUB_OPCODE_FOR_NAME.values()) + 1
        assert row < 0x20
        shas = {}
        for ver in ("v3", "v4"):
            tmp = DveOpSpec(name=name, opcode=row, uops=lower(spec, ver=ver),
                            rd1_en=_has_src1(spec))
            shas[ver] = tmp.sha(ver)
        op = dops.DveOp(name, spec, subdim=False, uops_sha=shas)
        dops.OPS.append(op)
        dops.CUSTOM_DVE_SPECS[name] = spec
        dops._SUB_OPCODE_FOR_NAME[name] = row
    dops._AT_FRAC = dops.OPS[-2]
    dops._BT_FUSE = dops.OPS[-1]


def _build():
    import concourse.mybir as mybir
    import concourse.tile as tile
    from concourse import bacc
    import concourse.dve_ops as dops

    _register_custom_ops()

    f32 = mybir.dt.float32
    bf16 = mybir.dt.bfloat16
    A = mybir.AluOpType
    Act = mybir.ActivationFunctionType

    nc = bacc.Bacc("TRN2", target_bir_lowering=False, debug=False, num_devices=B)

    xh_d = nc.dram_tensor("xh", [NT, 128, KD, TC], bf16, kind="ExternalInput")
    w_d = {p: nc.dram_tensor(f"w{p}", [KD, 128, H], bf16, kind="ExternalInput") for p in "fih"}
    b_d = {p: nc.dram_tensor(f"b{p}", [128, MH], f32, kind="ExternalInput") for p in "fih"}
    h0_d = nc.dram_tensor("h0c", [128, MH], f32, kind="ExternalInput")
    hT = nc.dram_tensor("hT", [H, T], bf16, kind="ExternalOutput")

    with tile.TileContext(nc) as tc:
        with (
            tc.tile_pool(name="wpool", bufs=1) as wpool,
            tc.tile_pool(name="cpool", bufs=1) as cpool,
            tc.tile_pool(name="xpool", bufs=2) as xpool,
            tc.tile_pool(name="pspool", bufs=8, space="PSUM") as pspool,
            tc.tile_pool(name="wk", bufs=4) as wk,
            tc.tile_pool(name="hpool", bufs=3) as hpool,
        ):
            # Head: only sync/scalar/gpsimd can issue DMAs. The PE clock
            # ramps (0.65 -> 1.2 -> 2.4GHz after 3us of CONTINUOUS busy)
            # and any stall resets the ramp. Feed the first group per-kd
            # so arrivals track the PE's ramp cadence, with only x (sync)
            # and wf (gpsimd) competing for HBM bandwidth early; b_f rides
            # just before x_kd5 so the first ACT isn't gated.
            xht0 = xpool.tile([128, KD, TC], bf16, tag="xh", name="xh_0")
            b_sb = {}
            for kd in range(5):
                nc.sync.dma_start(xht0[:, kd, :], xh_d[0, :, kd, :])
            b_sb["f"] = cpool.tile([128, MH], f32, tag="bf", name="bsf")
            nc.sync.dma_start(b_sb["f"][:], b_d["f"][:])
            nc.sync.dma_start(xht0[:, 5, :], xh_d[0, :, 5, :])
            for p in "ih":
                b_sb[p] = cpool.tile([128, MH], f32, tag=f"b{p}", name=f"bs{p}")
                nc.sync.dma_start(b_sb[p][:], b_d[p][:])
            h0_sb = cpool.tile([128, MH], f32, tag="h0")
            nc.sync.dma_start(h0_sb[:], h0_d[:])
            w_sb = {}
            w_q = {"f": nc.gpsimd, "i": nc.scalar, "h": nc.gpsimd}
            for p in "fih":
                w_sb[p] = wpool.tile([128, KD * H], bf16, tag=f"w{p}", name=f"w{p}s")
                for kd in range(KD):
                    w_q[p].dma_start(w_sb[p][:, kd * H:(kd + 1) * H], w_d[p][kd])

            prev_h = [None] * MH
            pending = []

            def emit_proj(c, j, ps, key, xht):
                # psf/psi drain at lag 0 (stage1 ACT); psh at lag<=2
                # (stage2's fused bt reads PSUM directly).
                tag, nb = ("psh", 3) if key == "h" else ("psfi", 5)
                pt = pspool.tile([128, TC], f32, tag=tag, bufs=nb, name=f"p{key}{c}_{j}")
                for kd in range(KD):
                    nc.tensor.matmul(
                        pt[:],
                        w_sb[key][:, kd * H + j * 128:kd * H + (j + 1) * 128],
                        xht[:, kd, :],
                        start=(kd == 0),
                        stop=(kd == KD - 1),
                    )
                ps[key] = pt

            def stage1f(c, j, ps):
                # f only needs psf -- drains the bank without waiting on wi.
                fs = wk.tile([128, TC], bf16, tag="f", name=f"f{c}_{j}")
                nc.scalar.activation(fs[:], ps["f"][:], Act.Sigmoid, bias=b_sb["f"][:, j:j + 1], scale=1.0)
                ps["fs"] = fs

            def stage1i(c, j, ps):
                si = wk.tile([128, TC], bf16, tag="i", name=f"i{c}_{j}")
                nc.scalar.activation(si[:], ps["i"][:], Act.Sigmoid, bias=b_sb["i"][:, j:j + 1], scale=1.0)
                pending.append((c, j, ps, ps["fs"], si))

            def stage2():
                c, j, ps, fs, si = pending.pop(0)
                at = wk.tile([128, TC], bf16, tag="a", name=f"at{c}_{j}")
                nc.vector._custom_dve(dops._AT_FRAC, out=at[:], in0=si[:], in1=fs[:],
                                      s0=_RC0, s1=_RC1)
                bt = wk.tile([128, TC], bf16, tag="b", name=f"bt{c}_{j}")
                nc.vector._custom_dve(dops._BT_FUSE, out=bt[:], in0=at[:], in1=ps["h"][:],
                                      s0=1.0, s1=b_sb["h"][:, j:j + 1])
                hh = hpool.tile([128, TC], bf16, tag=f"h{j}", name=f"hh{c}_{j}")
                init = h0_sb[:, j:j + 1] if c == 0 else prev_h[j][:, TC - 1:TC]
                nc.vector.tensor_tensor_scan(hh[:], at[:], bt[:], init, op0=A.mult, op1=A.add)
                prev_h[j] = hh
                nc.sync.dma_start(hT[j * 128:(j + 1) * 128, c * TC:(c + 1) * TC], hh[:])

            for c in range(NT):
                if c == 0:
                    xht = xht0
                else:
                    xht = xpool.tile([128, KD, TC], bf16, tag="xh", name=f"xh_{c}")
                    nc.sync.dma_start(xht[:], xh_d[c])

                ps_by_j = [dict() for _ in range(MH)]
                if c == 0:
                    # f-projections for all j first: they only need wf + x,
                    # streaming while wi/wh weight DMAs are still in flight;
                    # f drains each psf bank as it completes.
                    for j in range(MH):
                        emit_proj(c, j, ps_by_j[j], "f", xht)
                        stage1f(c, j, ps_by_j[j])
                for j in range(MH):
                    ps = ps_by_j[j]
                    if c != 0:
                        emit_proj(c, j, ps, "f", xht)
                        stage1f(c, j, ps)
                    emit_proj(c, j, ps, "i", xht)
                    emit_proj(c, j, ps, "h", xht)
                    stage1i(c, j, ps)
                    # Steady state: stage2 lags 2 groups so the in-order DVE
                    # queue never waits mid-chain; drain eagerly at the end.
                    lag = 1 if (c == NT - 1 and j >= MH - 2) else 2
                    while len(pending) > lag - 1:
                        stage2()
            while pending:
                stage2()

    # Keep every ACT func in one table (Sigmoid/Identity/Copy all live in
    # "sigmoid_and_others"); empty the other tables so the first-match
    # table-load pass emits a single load instead of thrashing.
    import concourse.bacc as bacc_mod

    orig_tables = bacc_mod.get_activation_tables

    def _single_table(arch):
        tabs = orig_tables(arch)
        keep = "sigmoid_and_others"
        return {k: (v if k == keep else set()) for k, v in tabs.items()}

    bacc_mod.get_activation_tables = _single_table
    try:
        nc.compile()
    finally:
        bacc_mod.get_activation_tables = orig_tables
    return nc


def _get_nc():
    if "nc" not in _state:
        _state["nc"] = _build()
    return _state["nc"]


def _prep_inputs(x, h0, f_w, f_b, i_w, i_b, h_w, h_b):
    BF = ml_dtypes.bfloat16
    x = np.asarray(x, dtype=np.float32)
    h0 = np.asarray(h0, dtype=np.float32)
    xT = x.transpose(0, 2, 1)                                # [B, D, T]
    # xh: [B, NT, 128, KD, TC] with k = kd*128 + kp
    xh = np.ascontiguousarray(
        xT.reshape(B, KD, 128, NT, TC).transpose(0, 3, 2, 1, 4)
    ).astype(BF)
    shared = {}
    for p, w in (("f", f_w), ("i", i_w), ("h", h_w)):
        wT = np.asarray(w, dtype=np.float32).T               # [D, H]
        shared[f"w{p}"] = np.ascontiguousarray(wT.reshape(KD, 128, H)).astype(BF)
    for p, bias in (("f", f_b), ("i", i_b), ("h", h_b)):
        bias = np.asarray(bias, dtype=np.float32)
        shared[f"b{p}"] = np.ascontiguousarray(bias.reshape(MH, 128).T)  # [128, MH]
    in_maps = []
    for b in range(B):
        m = dict(shared)
        m["xh"] = xh[b]
        m["h0c"] = np.ascontiguousarray(h0[b, 0].reshape(MH, 128).T)
        in_maps.append(m)
    return in_maps


def kernel(x, h0, f_w, f_b, i_w, i_b, h_w, h_b, _trace=False):
    from concourse.bass_utils import run_bass_kernel_spmd

    nc = _get_nc()
    in_maps = _prep_inputs(x, h0, f_w, f_b, i_w, i_b, h_w, h_b)
    res = run_bass_kernel_spmd(nc, in_maps, core_ids=list(range(B)), trace=_trace)
    out = np.empty((B, T, H), dtype=np.float32)
    for b in range(B):
        out[b] = res.results[b]["hT"].astype(np.float32).T
    if _trace:
        _state["last_results"] = res
    return out


# revision 13
# speedup vs baseline: 1.0340x; 1.0340x over previous
"""MinLSTM cell for Trainium2 (Bass/Tile), v5: sigmoid + fused custom-DVE.

Data-parallel over batch on 8 cores (one row per core). PE floor is
~187us (3 bf16 GEMMs at 1 row/cycle @2.4GHz; fp8 fails the 2e-2 gate:
measured 3-4e-2 in numpy). Everything else exists to keep PE streaming
at 216ns/matmul with zero backpressure.

v4 lesson (trace): the exp/ln gate chain was 5 hops deep
(ACT ef/ei -> Pool s2 -> ACT ln2/rt -> DVE ut/bt -> scan); PSUM banks
freed slowly through that chain, stalling the PE ~250ns at group
boundaries, and the tail drained at ~4.1us/group. v5 shortens the
chain to 2 hops with the identity fp+ip = 1:

  - ACT (2 ops/tile): f = Sigmoid(psf + bf), i = Sigmoid(psi + bi),
    both bf16, straight from PSUM (frees psf/psi at lag 0).
  - DVE (3 ops/tile): at = f/(f+i) as ONE fused op (bitwise-NOT
    exponent-flip reciprocal seed + 1 Newton step, 7 ALU stages,
    ~0.17% max err); bt = (1-at)*(psh+bh) as ONE fused op (3 stages,
    reads PSUM directly -- frees psh at lag 1, no identity-ACT pass);
    then the hw scan. at = fp exactly because fp+ip = 1.
  - Pool/GpSimd: only DMA issue + weight loads (was a serialization
    link in the chain; now out of the loop entirely).
  - output hT [H,T] bf16 (halves store traffic); host converts.
"""

import sys

if "/opt/trn_rl_repo" not in sys.path:
    sys.path.insert(0, "/opt/trn_rl_repo")

import numpy as np
import ml_dtypes

B, T, D, H = 8, 4096, 768, 768
TC = 512                    # time-chunk (one PSUM bank of fp32)
NT = T // TC                # 8 chunks
KD = D // 128               # 6 bf16 K-tiles
MH = H // 128               # 6 hidden tiles

# Chebyshev-minimax seed constants for x*bitcast(~x) in [-4.5, -4]
# (from concourse.dve_ops.RECIP_APPROX_FAST_CONSTS).
_RC0, _RC1 = -0.23549792, 2.0017324

_state = {}


def _register_custom_ops():
    """Register the two fused DVE ops in concourse.dve_ops' registry so
    dve_table_for_ops can build the per-NEFF table for them."""
    import concourse.dve_ops as dops
    from concourse.dve_spec import Spec, Src0, Src1, C0, C1, AluOp, Bin, lower, _has_src1
    from concourse.dve_uop import DveOpSpec

    if "AT_FRAC_ANT" in dops._SUB_OPCODE_FOR_NAME:
        return

    def _recip1(x):
        x = x.astype(np.float32)
        nx = (~x.view(np.int32)).view(np.float32)
        y0 = (nx * np.float32(_RC0)).astype(np.float32)
        return (y0 * (np.float32(_RC1) - (x * y0).astype(np.float32))).astype(np.float32)

    # at = Src1 / (Src0 + Src1)  (call with in0=i, in1=f -> at = f/(f+i))
    x = Src0 + Src1
    nx = Bin(AluOp.BITWISE_NOT, x, x)
    y0 = nx * C0
    y1 = y0 * (C1 - x * y0)
    at_spec = Spec(
        body=Src1 * y1,
        reference=lambda in0, in1, s0, s1, imm2: (
            in1.astype(np.float32) * _recip1(in0.astype(np.float32) + in1)
        ),
    )
    # bt = (C0 - Src0) * (Src1 + C1)  (in0=at, in1=psh, s0=1.0, s1=bh)
    bt_spec = Spec(
        body=(C0 - Src0) * (Src1 + C1),
        reference=lambda in0, in1, s0, s1, imm2: (
            (s0 - in0.astype(np.float32)) * (in1 + s1)
        ),
    )

    for name, spec in (("AT_FRAC_ANT", at_spec), ("BT_FUSE_ANT", bt_spec)):
        row = max(dops._SUB_OPCODE_FOR_NAME.values()) + 1
        assert row < 0x20
        shas = {}
        for ver in ("v3", "v4"):
            tmp = DveOpSpec(name=name, opcode=row, uops=lower(spec, ver=ver),
                            rd1_en=_has_src1(spec))
            shas[ver] = tmp.sha(ver)
        op = dops.DveOp(name, spec, subdim=False, uops_sha=shas)
        dops.OPS.append(op)
        dops.CUSTOM_DVE_SPECS[name] = spec
        dops._SUB_OPCODE_FOR_NAME[name] = row
    dops._AT_FRAC = dops.OPS[-2]
    dops._BT_FUSE = dops.OPS[-1]


def _build():
    import concourse.mybir as mybir
    import concourse.tile as tile
    from concourse import bacc
    import concourse.dve_ops as dops

    _register_custom_ops()

    f32 = mybir.dt.float32
    bf16 = mybir.dt.bfloat16
    A = mybir.AluOpType
    Act = mybir.ActivationFunctionType

    nc = bacc.Bacc("TRN2", target_bir_lowering=False, debug=False, num_devices=B)

    xh_d = nc.dram_tensor("xh", [NT, 128, KD, TC], bf16, kind="ExternalInput")
    w_d = {p: nc.dram_tensor(f"w{p}", [KD, 128, H], bf16, kind="ExternalInput") for p in "fih"}
    b_d = {p: nc.dram_tensor(f"b{p}", [128, MH], f32, kind="ExternalInput") for p in "fih"}
    h0_d = nc.dram_tensor("h0c", [128, MH], f32, kind="ExternalInput")
    hT = nc.dram_tensor("hT", [H, T], bf16, kind="ExternalOutput")

    with tile.TileContext(nc) as tc:
        with (
            tc.tile_pool(name="wpool", bufs=1) as wpool,
            tc.tile_pool(name="cpool", bufs=1) as cpool,
            tc.tile_pool(name="xpool", bufs=2) as xpool,
            tc.tile_pool(name="pspool", bufs=8, space="PSUM") as pspool,
            tc.tile_pool(name="wk", bufs=4) as wk,
            tc.tile_pool(name="hpool", bufs=3) as hpool,
        ):
            # Head: only sync/scalar/gpsimd can issue DMAs. The PE clock
            # ramps (0.65 -> 1.2 -> 2.4GHz after 3us of CONTINUOUS busy)
            # and any stall resets the ramp. c0's f-projections run as two
            # K-passes (kd0-2 all j, then kd3-5), so the PE only needs
            # x_kd0-2 + wf_kd0-2 (~1MB) to start and run gapless while
            # the rest lands. Queues: sync = x_kd0-2, biases, wh, x_c1+;
            # scalar = x_kd3-5 then wi; gpsimd = wf alone.
            xht0 = xpool.tile([128, KD, TC], bf16, tag="xh", name="xh_0")
            b_sb = {}
            for kd in range(3):
                nc.sync.dma_start(xht0[:, kd, :], xh_d[0, :, kd, :])
            for kd in range(3, KD):
                nc.scalar.dma_start(xht0[:, kd, :], xh_d[0, :, kd, :])
            w_sb = {}
            for p in "fih":
                w_sb[p] = wpool.tile([128, KD * H], bf16, tag=f"w{p}", name=f"w{p}s")
            for kd in range(KD):
                nc.gpsimd.dma_start(w_sb["f"][:, kd * H:(kd + 1) * H], w_d["f"][kd])
            for p in "fih":
                b_sb[p] = cpool.tile([128, MH], f32, tag=f"b{p}", name=f"bs{p}")
                nc.sync.dma_start(b_sb[p][:], b_d[p][:])
            h0_sb = cpool.tile([128, MH], f32, tag="h0")
            nc.sync.dma_start(h0_sb[:], h0_d[:])
            for kd in range(KD):
                nc.scalar.dma_start(w_sb["i"][:, kd * H:(kd + 1) * H], w_d["i"][kd])
            for kd in range(KD):
                nc.sync.dma_start(w_sb["h"][:, kd * H:(kd + 1) * H], w_d["h"][kd])

            prev_h = [None] * MH
            pending = []

            def emit_proj(c, j, ps, key, xht):
                # psf/psi drain at lag 0 (stage1 ACT); psh at lag<=2
                # (stage2's fused bt reads PSUM directly).
                tag, nb = ("psh", 3) if key == "h" else ("psfi", 5)
                pt = pspool.tile([128, TC], f32, tag=tag, bufs=nb, name=f"p{key}{c}_{j}")
                for kd in range(KD):
                    nc.tensor.matmul(
                        pt[:],
                        w_sb[key][:, kd * H + j * 128:kd * H + (j + 1) * 128],
                        xht[:, kd, :],
                        start=(kd == 0),
                        stop=(kd == KD - 1),
                    )
                ps[key] = pt

            def stage1f(c, j, ps):
                # f only needs psf -- drains the bank without waiting on wi.
                fs = wk.tile([128, TC], bf16, tag="f", name=f"f{c}_{j}")
                nc.scalar.activation(fs[:], ps["f"][:], Act.Sigmoid, bias=b_sb["f"][:, j:j + 1], scale=1.0)
                ps["fs"] = fs

            def stage1i(c, j, ps):
                si = wk.tile([128, TC], bf16, tag="i", name=f"i{c}_{j}")
                nc.scalar.activation(si[:], ps["i"][:], Act.Sigmoid, bias=b_sb["i"][:, j:j + 1], scale=1.0)
                pending.append((c, j, ps, ps["fs"], si))

            def stage2():
                c, j, ps, fs, si = pending.pop(0)
                at = wk.tile([128, TC], bf16, tag="a", name=f"at{c}_{j}")
                nc.vector._custom_dve(dops._AT_FRAC, out=at[:], in0=si[:], in1=fs[:],
                                      s0=_RC0, s1=_RC1)
                bt = wk.tile([128, TC], bf16, tag="b", name=f"bt{c}_{j}")
                nc.vector._custom_dve(dops._BT_FUSE, out=bt[:], in0=at[:], in1=ps["h"][:],
                                      s0=1.0, s1=b_sb["h"][:, j:j + 1])
                hh = hpool.tile([128, TC], bf16, tag=f"h{j}", name=f"hh{c}_{j}")
                init = h0_sb[:, j:j + 1] if c == 0 else prev_h[j][:, TC - 1:TC]
                nc.vector.tensor_tensor_scan(hh[:], at[:], bt[:], init, op0=A.mult, op1=A.add)
                prev_h[j] = hh
                nc.sync.dma_start(hT[j * 128:(j + 1) * 128, c * TC:(c + 1) * TC], hh[:])

            for c in range(NT):
                if c == 0:
                    xht = xht0
                else:
                    xht = xpool.tile([128, KD, TC], bf16, tag="xh", name=f"xh_{c}")
                    nc.sync.dma_start(xht[:], xh_d[c])

                ps_by_j = [dict() for _ in range(MH)]
                if c == 0:
                    # f-projections for all j in two K-passes: pass 1 needs
                    # only x_kd0-2 + wf_kd0-2; pass 2's weights land while
                    # pass 1 streams. j5 borrows a psh-tag bank (6 concurrent
                    # f accumulations; psh isn't otherwise used yet).
                    for j in range(MH):
                        tag, nb = ("psfi", 5) if j < 5 else ("psh", 3)
                        pt = pspool.tile([128, TC], f32, tag=tag, bufs=nb, name=f"pf0_{j}")
                        ps_by_j[j]["f"] = pt
                        for kd in range(3):
                            nc.tensor.matmul(
                                pt[:],
                                w_sb["f"][:, kd * H + j * 128:kd * H + (j + 1) * 128],
                                xht[:, kd, :],
                                start=(kd == 0),
                                stop=False,
                            )
                    for j in range(MH):
                        pt = ps_by_j[j]["f"]
                        for kd in range(3, KD):
                            nc.tensor.matmul(
                                pt[:],
                                w_sb["f"][:, kd * H + j * 128:kd * H + (j + 1) * 128],
                                xht[:, kd, :],
                                start=False,
                                stop=(kd == KD - 1),
                            )
                        stage1f(c, j, ps_by_j[j])
                for j in range(MH):
                    ps = ps_by_j[j]
                    if c != 0:
                        emit_proj(c, j, ps, "f", xht)
                        stage1f(c, j, ps)
                    emit_proj(c, j, ps, "i", xht)
                    emit_proj(c, j, ps, "h", xht)
                    stage1i(c, j, ps)
                    # Steady state: stage2 lags 2 groups so the in-order DVE
                    # queue never waits mid-chain; drain eagerly at the end.
                    lag = 1 if (c == NT - 1 and j >= MH - 2) else 2
                    while len(pending) > lag - 1:
                        stage2()
            while pending:
                stage2()

    # Keep every ACT func in one table (Sigmoid/Identity/Copy all live in
    # "sigmoid_and_others"); empty the other tables so the first-match
    # table-load pass emits a single load instead of thrashing.
    import concourse.bacc as bacc_mod

    orig_tables = bacc_mod.get_activation_tables

    def _single_table(arch):
        tabs = orig_tables(arch)
        keep = "sigmoid_and_others"
        return {k: (v if k == keep else set()) for k, v in tabs.items()}

    bacc_mod.get_activation_tables = _single_table
    try:
        nc.compile()
    finally:
        bacc_mod.get_activation_tables = orig_tables
    return nc


def _get_nc():
    if "nc" not in _state:
        _state["nc"] = _build()
    return _state["nc"]


def _prep_inputs(x, h0, f_w, f_b, i_w, i_b, h_w, h_b):
    BF = ml_dtypes.bfloat16
    x = np.asarray(x, dtype=np.float32)
    h0 = np.asarray(h0, dtype=np.float32)
    xT = x.transpose(0, 2, 1)                                # [B, D, T]
    # xh: [B, NT, 128, KD, TC] with k = kd*128 + kp
    xh = np.ascontiguousarray(
        xT.reshape(B, KD, 128, NT, TC).transpose(0, 3, 2, 1, 4)
    ).astype(BF)
    shared = {}
    for p, w in (("f", f_w), ("i", i_w), ("h", h_w)):
        wT = np.asarray(w, dtype=np.float32).T               # [D, H]
        shared[f"w{p}"] = np.ascontiguousarray(wT.reshape(KD, 128, H)).astype(BF)
    for p, bias in (("f", f_b), ("i", i_b), ("h", h_b)):
        bias = np.asarray(bias, dtype=np.float32)
        shared[f"b{p}"] = np.ascontiguousarray(bias.reshape(MH, 128).T)  # [128, MH]
    in_maps = []
    for b in range(B):
        m = dict(shared)
        m["xh"] = xh[b]
        m["h0c"] = np.ascontiguousarray(h0[b, 0].reshape(MH, 128).T)
        in_maps.append(m)
    return in_maps


def kernel(x, h0, f_w, f_b, i_w, i_b, h_w, h_b, _trace=False):
    from concourse.bass_utils import run_bass_kernel_spmd

    nc = _get_nc()
    in_maps = _prep_inputs(x, h0, f_w, f_b, i_w, i_b, h_w, h_b)
    res = run_bass_kernel_spmd(nc, in_maps, core_ids=list(range(B)), trace=_trace)
    out = np.empty((B, T, H), dtype=np.float32)
    for b in range(B):
        out[b] = res.results[b]["hT"].astype(np.float32).T
    if _trace:
        _state["last_results"] = res
    return out
